# revision 6
# baseline (speedup 1.0000x reference)
"""Trainium2 Bass kernel for (W0 (x) W1 (x) W2 (x) W3) @ x  -- Kronecker chain.

v3 over baseline (96167 -> 88189 ns in the TRN2 timeline model):
- packed weights: one DMA instead of four (startup)
- per-th output stores (tail drains after the last 1024-col evac, not 4096)
- PE p-state warmup: two zero matmuls ramp the tensor engine to 2.4 GHz
  before the first input chunk lands
- evac engine assignment: fixed roles -- chained evacs (S1, S3) on DVE,
  terminal evacs (S2, S4) on Act -- with three S3 evacs skewed to Act at
  empirically chosen positions; all finer-grained skews/splits lose to
  scheduler convoys and per-op fixed costs
"""
import numpy as np
import ml_dtypes

import concourse.bass as bass
import concourse.bacc as bacc
import concourse.mybir as mybir
import concourse.tile as tile
from concourse.bass_utils import run_bass_kernel_spmd

F32 = mybir.dt.float32
BF16 = mybir.dt.bfloat16

L = 32
N = L ** 4          # 1048576
B = 32
NCORES = 8
BC = B // NCORES    # 4

_NC_CACHE = {}


def _build_nc():
    nc = bacc.Bacc("TRN2", target_bir_lowering=False, debug=False)

    # x pre-shuffled on host to [j0, (j1h, j2), (j1l, j3, b)], bf16
    x = nc.dram_tensor("x", [32, 131072], BF16, kind="ExternalInput")
    # augmented weights side by side: [128, (w2|w3|w1|w0) 512]
    wpack = nc.dram_tensor("wpack", [128, 512], BF16, kind="ExternalInput")
    # y device order: [i2a(8), (i0, i3b)(128), (i3a, b, i1, i2b)(4096)] bf16
    y = nc.dram_tensor("y", [8, 524288], BF16, kind="ExternalOutput")

    def evac(eng, out_ap, in_ap):
        if eng == 'a':
            nc.scalar.copy(out=out_ap, in_=in_ap)
        else:
            nc.vector.tensor_copy(out_ap, in_ap)

    with tile.TileContext(nc) as tc:
        with tc.tile_pool(name="wp", bufs=1) as wp, \
             tc.tile_pool(name="zp", bufs=1) as zp, \
             tc.tile_pool(name="b1p", bufs=1) as b1p:
            ws = wp.tile([128, 512], BF16, name="ws")
            nc.sync.dma_start(out=ws[:], in_=wpack.ap())
            w2s, w3s = ws[:, 0:128], ws[:, 128:256]
            w1s, w0s = ws[:, 256:384], ws[:, 384:512]

            # PE p-state warmup: zero matmuls ramp the tensor engine to
            # full clock before the first real matmul's data arrives.
            zt = zp.tile([128, 512], BF16, name="zt")
            nc.vector.memset(zt[:], 0)
            # B1: [part (i2b,j1), addr = i2a*4096 + i3a*512 + b*128 + i3b*32 + j0] bf16
            b1 = b1p.tile([128, 32768], BF16, name="b1")
            b1_t, b1_o = b1.tensor, b1.offset

            # ---- Phase I: S1 (contract j2) + S2 (contract j3), per j0 ----
            with tc.tile_pool(name="lp", bufs=4) as lp, \
                 tc.tile_pool(name="t1p", bufs=4) as t1p, \
                 tc.tile_pool(name="ps1", bufs=2, space="PSUM") as ps1, \
                 tc.tile_pool(name="ps2", bufs=2, space="PSUM") as ps2:
                p1_pre = ps1.tile([128, 1024], F32, name="p1")
                for d in range(2):
                    nc.tensor.matmul(p1_pre[:, 0:512], zt[:, 0:128], zt[:],
                                     start=True, stop=True)
                for j0 in range(32):
                    if True:
                        lt = lp.tile([128, 1024], BF16, name="lt")
                        nc.sync.dma_start(
                            out=lt[:],
                            in_=bass.AP(x, j0 * 131072, [[1024, 128], [1, 1024]]))
                        halves = [(lt.tensor, lt.offset),
                                  (lt.tensor, lt.offset + 512)]

                    # T1: [part (j3, b), free addr = i2a*128+i2b*32+j1h*8+j1l]
                    t1 = t1p.tile([128, 1024], BF16, name="t1")
                    t1_t, t1_o = t1.tensor, t1.offset
                    p1 = p1_pre if j0 == 0 else ps1.tile([128, 1024], F32, name="p1")
                    for j1l in range(8):
                        ht, ho = halves[j1l // 4]
                        lhsT = bass.AP(ht, ho + (j1l % 4) * 128,
                                       [[1024, 128], [1, 128]])
                        nc.tensor.matmul(p1[:, j1l * 128:(j1l + 1) * 128],
                                         lhsT, w2s, start=True, stop=True)
                    # psum pos (j1l, n1=(i2a,i2b,j1h)); merge (i2b,j1h)->[8,16]
                    # Fixed roles: chained S1 evacs on DVE, terminal S2 evacs
                    # on Act (measured optimal vs alternation under the
                    # warmed-up schedule).
                    e1, e2 = ('d', 'a')
                    evac(e1, bass.AP(t1_t, t1_o,
                                     [[1024, 128], [1, 8], [128, 8], [8, 16]]),
                         p1[:])

                    p2 = ps2.tile([128, 1024], F32, name="p2")
                    for i2a in range(8):
                        lhsT = bass.AP(t1_t, t1_o + i2a * 128,
                                       [[1024, 128], [1, 128]])
                        nc.tensor.matmul(p2[:, i2a * 128:(i2a + 1) * 128],
                                         lhsT, w3s, start=True, stop=True)
                    # psum pos (i2a, n2=(i3a,b,i3b)); merge (b,i3b)->[32,16]
                    evac(e2, bass.AP(b1_t, b1_o + j0,
                                     [[32768, 128], [4096, 8], [512, 8], [32, 16]]),
                         p2[:])

            # ---- Phase II: S3 (contract j1) + S4 (contract j0), per i2a ----
            with tc.tile_pool(name="t3p", bufs=4) as t3p, \
                 tc.tile_pool(name="stgp", bufs=3) as stgp, \
                 tc.tile_pool(name="ps3", bufs=2, space="PSUM") as ps3, \
                 tc.tile_pool(name="ps4", bufs=2, space="PSUM") as ps4:
                for k in range(8):  # k = i2a
                    # T3: [part (i3b,j0), free (i3a:512, b:128, (i1*4+i2b):1)]
                    t3 = t3p.tile([128, 4096], BF16, name="t3")
                    t3_t, t3_o = t3.tensor, t3.offset
                    for th in range(4):  # pairs of i3a
                        p3 = ps3.tile([128, 1024], F32, name="p3")
                        for q in range(8):
                            cq = 8 * th + q      # cq = i3a*4 + b
                            lhsT = bass.AP(b1_t,
                                           b1_o + k * 4096 + cq * 128,
                                           [[32768, 128], [1, 128]])
                            nc.tensor.matmul(p3[:, q * 128:(q + 1) * 128],
                                             lhsT, w1s, start=True, stop=True)
                        # S3 evacs on DVE except three positions skewed to
                        # Act (empirically best load-balance points).
                        evac('a' if (4 * k + th) in (2, 10, 22) else 'd',
                             t3[:, th * 1024:(th + 1) * 1024], p3[:])

                    stg = stgp.tile([128, 4096], BF16, name="stg")
                    stg_t, stg_o = stg.tensor, stg.offset
                    for th in range(4):  # pairs of i3a
                        p4 = ps4.tile([128, 1024], F32, name="p4")
                        for m in range(2):
                            i3a = 2 * th + m
                            rhs = bass.AP(t3_t, t3_o + i3a * 512,
                                          [[4096, 128], [128, 4], [1, 128]])
                            nc.tensor.matmul(p4[:, m * 512:(m + 1) * 512],
                                             w0s, rhs, start=True, stop=True)
                        evac('a', stg[:, th * 1024:(th + 1) * 1024], p4[:])
                        nc.sync.dma_start(
                            out=bass.AP(y, k * 524288 + th * 1024,
                                        [[4096, 128], [1, 1024]]),
                            in_=bass.AP(stg_t, stg_o + th * 1024,
                                        [[4096, 128], [1, 1024]]))

    nc.finalize()
    return nc


def _build_waug(w: np.ndarray, kind: str) -> np.ndarray:
    """Augmented 128x128 weights (see baseline docstring)."""
    wa = np.zeros((128, 128), dtype=np.float32)
    ar = np.arange(32)
    if kind == "w3":
        # rows p = j3*4 + b ; cols n = i3a*16 + b*4 + i3b
        for b in range(4):
            cols = (ar >> 2) * 16 + b * 4 + (ar & 3)
            wa[np.ix_(ar * 4 + b, cols)] = w.T
    else:
        # rows p = q*32 + j ; cols n = i*4 + q
        for q in range(4):
            wa[np.ix_(q * 32 + ar, ar * 4 + q)] = w.T
    return wa


def _get_nc():
    if "nc" not in _NC_CACHE:
        _NC_CACHE["nc"] = _build_nc()
    return _NC_CACHE["nc"]


def make_in_maps(x, W0, W1, W2, W3):
    x = np.asarray(x, dtype=np.float32)
    bf = ml_dtypes.bfloat16
    wpack = np.concatenate([
        _build_waug(np.asarray(W2, np.float32), "q"),
        _build_waug(np.asarray(W3, np.float32), "w3"),
        _build_waug(np.asarray(W1, np.float32), "q"),
        _build_waug(np.asarray(W0, np.float32), "q"),
    ], axis=1).astype(bf)
    xr = x.reshape(32, 4, 8, 32, 32, B)
    in_maps = []
    for c in range(NCORES):
        xc = xr[..., c * BC:(c + 1) * BC].transpose(0, 1, 3, 2, 4, 5)
        xc = np.ascontiguousarray(xc).astype(bf).reshape(32, 131072)
        in_maps.append({"x": xc, "wpack": wpack})
    return in_maps


def _unshuffle_y(yd: np.ndarray) -> np.ndarray:
    """[i2a(8), (i0, i3b), (i3a, b, i1, i2b)] -> [N, BC]."""
    y = yd.astype(np.float32).reshape(8, 32, 4, 8, BC, 32, 4)
    y = y.transpose(1, 5, 0, 6, 3, 2, 4)
    return np.ascontiguousarray(y).reshape(N, BC)


def kernel(x, W0, W1, W2, W3, _trace=False):
    nc = _get_nc()
    in_maps = make_in_maps(x, W0, W1, W2, W3)
    res = run_bass_kernel_spmd(nc, in_maps, core_ids=list(range(NCORES)),
                               trace=_trace)
    out = np.concatenate(
        [_unshuffle_y(res.results[c]["y"]) for c in range(NCORES)], axis=1)
    if _trace:
        kernel.last_result = res
    return out


if __name__ == "__main__":
    rng = np.random.default_rng(0)
    x = rng.standard_normal((N, B), dtype=np.float32)
    ws = [rng.standard_normal((L, L), dtype=np.float32) for _ in range(4)]
    y = kernel(x, *ws)
    print("ran", y.shape, y.dtype)


# revision 9
# speedup vs baseline: 1.0044x; 1.0044x over previous
"""Trainium2 Bass kernel for (W0 (x) W1 (x) W2 (x) W3) @ x  -- Kronecker chain.

v3 over baseline (96167 -> 87803 ns in the TRN2 timeline model):
- boot DMA: augmented weights + the j0=0 input chunk in a single transfer,
  removing one DMA round-trip from the startup critical path
- per-th output stores (tail drains after the last 1024-col evac, not 4096)
- PE p-state warmup: two zero matmuls ramp the tensor engine to 2.4 GHz
  before the first input chunk lands
- evac engine assignment: fixed roles -- chained evacs (S1, S3) on DVE,
  terminal evacs (S2, S4) on Act -- with three S3 evacs skewed to Act at
  empirically chosen positions; all finer-grained skews/splits lose to
  scheduler convoys and per-op fixed costs
"""
import numpy as np
import ml_dtypes

import concourse.bass as bass
import concourse.bacc as bacc
import concourse.mybir as mybir
import concourse.tile as tile
from concourse.bass_utils import run_bass_kernel_spmd

F32 = mybir.dt.float32
BF16 = mybir.dt.bfloat16

L = 32
N = L ** 4          # 1048576
B = 32
NCORES = 8
BC = B // NCORES    # 4

_NC_CACHE = {}


def _build_nc():
    nc = bacc.Bacc("TRN2", target_bir_lowering=False, debug=False)

    # x pre-shuffled on host to [j0, (j1h, j2), (j1l, j3, b)], bf16
    x = nc.dram_tensor("x", [32, 131072], BF16, kind="ExternalInput")
    # boot block: augmented weights (w2|w3|w1|w0, 512 cols) followed by
    # the j0=0 input chunk (1024 cols) so one DMA covers the whole startup
    # critical path.
    wpack = nc.dram_tensor("wpack", [128, 1536], BF16, kind="ExternalInput")
    # y device order: [i2a(8), (i0, i3b)(128), (i3a, b, i1, i2b)(4096)] bf16
    y = nc.dram_tensor("y", [8, 524288], BF16, kind="ExternalOutput")

    def evac(eng, out_ap, in_ap):
        if eng == 'a':
            nc.scalar.copy(out=out_ap, in_=in_ap)
        else:
            nc.vector.tensor_copy(out_ap, in_ap)

    with tile.TileContext(nc) as tc:
        with tc.tile_pool(name="wp", bufs=1) as wp, \
             tc.tile_pool(name="zp", bufs=1) as zp, \
             tc.tile_pool(name="b1p", bufs=1) as b1p:
            ws = wp.tile([128, 1536], BF16, name="ws")
            nc.sync.dma_start(out=ws[:], in_=wpack.ap())
            boot_x = (ws.tensor, ws.offset + 512)
            w2s, w3s = ws[:, 0:128], ws[:, 128:256]
            w1s, w0s = ws[:, 256:384], ws[:, 384:512]

            # PE p-state warmup: zero matmuls ramp the tensor engine to
            # full clock before the first real matmul's data arrives.
            zt = zp.tile([128, 512], BF16, name="zt")
            nc.vector.memset(zt[:], 0)
            # B1: [part (i2b,j1), addr = i2a*4096 + i3a*512 + b*128 + i3b*32 + j0] bf16
            b1 = b1p.tile([128, 32768], BF16, name="b1")
            b1_t, b1_o = b1.tensor, b1.offset

            # ---- Phase I: S1 (contract j2) + S2 (contract j3), per j0 ----
            with tc.tile_pool(name="lp", bufs=4) as lp, \
                 tc.tile_pool(name="t1p", bufs=4) as t1p, \
                 tc.tile_pool(name="ps1", bufs=2, space="PSUM") as ps1, \
                 tc.tile_pool(name="ps2", bufs=2, space="PSUM") as ps2:
                p1_pre = ps1.tile([128, 1024], F32, name="p1")
                for d in range(2):
                    nc.tensor.matmul(p1_pre[:, 0:512], zt[:, 0:128], zt[:],
                                     start=True, stop=True)
                for j0 in range(32):
                    if j0 > 0:
                        lt = lp.tile([128, 1024], BF16, name="lt")
                        nc.sync.dma_start(
                            out=lt[:],
                            in_=bass.AP(x, j0 * 131072, [[1024, 128], [1, 1024]]))
                        halves = [(lt.tensor, lt.offset, 1024),
                                  (lt.tensor, lt.offset + 512, 1024)]
                    else:
                        # boot rows are 1536 wide; partition step must match
                        halves = [(boot_x[0], boot_x[1], 1536),
                                  (boot_x[0], boot_x[1] + 512, 1536)]

                    # T1: [part (j3, b), free addr = i2a*128+i2b*32+j1h*8+j1l]
                    t1 = t1p.tile([128, 1024], BF16, name="t1")
                    t1_t, t1_o = t1.tensor, t1.offset
                    p1 = p1_pre if j0 == 0 else ps1.tile([128, 1024], F32, name="p1")
                    for j1l in range(8):
                        ht, ho, hp = halves[j1l // 4]
                        lhsT = bass.AP(ht, ho + (j1l % 4) * 128,
                                       [[hp, 128], [1, 128]])
                        nc.tensor.matmul(p1[:, j1l * 128:(j1l + 1) * 128],
                                         lhsT, w2s, start=True, stop=True)
                    # psum pos (j1l, n1=(i2a,i2b,j1h)); merge (i2b,j1h)->[8,16]
                    # Fixed roles: chained S1 evacs on DVE, terminal S2 evacs
                    # on Act (measured optimal vs alternation under the
                    # warmed-up schedule).
                    e1, e2 = ('d', 'a')
                    evac(e1, bass.AP(t1_t, t1_o,
                                     [[1024, 128], [1, 8], [128, 8], [8, 16]]),
                         p1[:])

                    p2 = ps2.tile([128, 1024], F32, name="p2")
                    for i2a in range(8):
                        lhsT = bass.AP(t1_t, t1_o + i2a * 128,
                                       [[1024, 128], [1, 128]])
                        nc.tensor.matmul(p2[:, i2a * 128:(i2a + 1) * 128],
                                         lhsT, w3s, start=True, stop=True)
                    # psum pos (i2a, n2=(i3a,b,i3b)); merge (b,i3b)->[32,16]
                    evac(e2, bass.AP(b1_t, b1_o + j0,
                                     [[32768, 128], [4096, 8], [512, 8], [32, 16]]),
                         p2[:])

            # ---- Phase II: S3 (contract j1) + S4 (contract j0), per i2a ----
            with tc.tile_pool(name="t3p", bufs=4) as t3p, \
                 tc.tile_pool(name="stgp", bufs=3) as stgp, \
                 tc.tile_pool(name="ps3", bufs=2, space="PSUM") as ps3, \
                 tc.tile_pool(name="ps4", bufs=2, space="PSUM") as ps4:
                for k in range(8):  # k = i2a
                    # T3: [part (i3b,j0), free (i3a:512, b:128, (i1*4+i2b):1)]
                    t3 = t3p.tile([128, 4096], BF16, name="t3")
                    t3_t, t3_o = t3.tensor, t3.offset
                    for th in range(4):  # pairs of i3a
                        p3 = ps3.tile([128, 1024], F32, name="p3")
                        for q in range(8):
                            cq = 8 * th + q      # cq = i3a*4 + b
                            lhsT = bass.AP(b1_t,
                                           b1_o + k * 4096 + cq * 128,
                                           [[32768, 128], [1, 128]])
                            nc.tensor.matmul(p3[:, q * 128:(q + 1) * 128],
                                             lhsT, w1s, start=True, stop=True)
                        # S3 evacs on DVE except three positions skewed to
                        # Act (empirically best load-balance points).
                        evac('a' if (4 * k + th) in (2, 10, 22) else 'd',
                             t3[:, th * 1024:(th + 1) * 1024], p3[:])

                    stg = stgp.tile([128, 4096], BF16, name="stg")
                    stg_t, stg_o = stg.tensor, stg.offset
                    for th in range(4):  # pairs of i3a
                        p4 = ps4.tile([128, 1024], F32, name="p4")
                        for m in range(2):
                            i3a = 2 * th + m
                            rhs = bass.AP(t3_t, t3_o + i3a * 512,
                                          [[4096, 128], [128, 4], [1, 128]])
                            nc.tensor.matmul(p4[:, m * 512:(m + 1) * 512],
                                             w0s, rhs, start=True, stop=True)
                        evac('a', stg[:, th * 1024:(th + 1) * 1024], p4[:])
                        nc.sync.dma_start(
                            out=bass.AP(y, k * 524288 + th * 1024,
                                        [[4096, 128], [1, 1024]]),
                            in_=bass.AP(stg_t, stg_o + th * 1024,
                                        [[4096, 128], [1, 1024]]))

    nc.finalize()
    return nc


def _build_waug(w: np.ndarray, kind: str) -> np.ndarray:
    """Augmented 128x128 weights (see baseline docstring)."""
    wa = np.zeros((128, 128), dtype=np.float32)
    ar = np.arange(32)
    if kind == "w3":
        # rows p = j3*4 + b ; cols n = i3a*16 + b*4 + i3b
        for b in range(4):
            cols = (ar >> 2) * 16 + b * 4 + (ar & 3)
            wa[np.ix_(ar * 4 + b, cols)] = w.T
    else:
        # rows p = q*32 + j ; cols n = i*4 + q
        for q in range(4):
            wa[np.ix_(q * 32 + ar, ar * 4 + q)] = w.T
    return wa


def _get_nc():
    if "nc" not in _NC_CACHE:
        _NC_CACHE["nc"] = _build_nc()
    return _NC_CACHE["nc"]


def make_in_maps(x, W0, W1, W2, W3):
    x = np.asarray(x, dtype=np.float32)
    bf = ml_dtypes.bfloat16
    wblock = np.concatenate([
        _build_waug(np.asarray(W2, np.float32), "q"),
        _build_waug(np.asarray(W3, np.float32), "w3"),
        _build_waug(np.asarray(W1, np.float32), "q"),
        _build_waug(np.asarray(W0, np.float32), "q"),
    ], axis=1).astype(bf)
    xr = x.reshape(32, 4, 8, 32, 32, B)
    in_maps = []
    for c in range(NCORES):
        xc = xr[..., c * BC:(c + 1) * BC].transpose(0, 1, 3, 2, 4, 5)
        xc = np.ascontiguousarray(xc).astype(bf).reshape(32, 131072)
        wpack = np.concatenate([wblock, xc[0].reshape(128, 1024)], axis=1)
        in_maps.append({"x": xc, "wpack": wpack})
    return in_maps


def _unshuffle_y(yd: np.ndarray) -> np.ndarray:
    """[i2a(8), (i0, i3b), (i3a, b, i1, i2b)] -> [N, BC]."""
    y = yd.astype(np.float32).reshape(8, 32, 4, 8, BC, 32, 4)
    y = y.transpose(1, 5, 0, 6, 3, 2, 4)
    return np.ascontiguousarray(y).reshape(N, BC)


def kernel(x, W0, W1, W2, W3, _trace=False):
    nc = _get_nc()
    in_maps = make_in_maps(x, W0, W1, W2, W3)
    res = run_bass_kernel_spmd(nc, in_maps, core_ids=list(range(NCORES)),
                               trace=_trace)
    out = np.concatenate(
        [_unshuffle_y(res.results[c]["y"]) for c in range(NCORES)], axis=1)
    if _trace:
        kernel.last_result = res
    return out


if __name__ == "__main__":
    rng = np.random.default_rng(0)
    x = rng.standard_normal((N, B), dtype=np.float32)
    ws = [rng.standard_normal((L, L), dtype=np.float32) for _ in range(4)]
    y = kernel(x, *ws)
    print("ran", y.shape, y.dtype)


# revision 10
# speedup vs baseline: 1.0151x; 1.0107x over previous
"""Trainium2 Bass kernel for (W0 (x) W1 (x) W2 (x) W3) @ x  -- Kronecker chain.

v3 over baseline (96167 -> 86876 ns in the TRN2 timeline model):
- boot DMA: augmented weights + the j0=0 input chunk in a single transfer,
  removing one DMA round-trip from the startup critical path
- per-th output stores (tail drains after the last 1024-col evac, not 4096)
- PE p-state warmup: two zero matmuls ramp the tensor engine to 2.4 GHz
  before the first input chunk lands
- evac engine assignment: fixed roles -- chained evacs (S1, S3) on DVE,
  terminal evacs (S2, S4) on Act -- with three S3 evacs skewed to Act at
  empirically chosen positions; all finer-grained skews/splits lose to
  scheduler convoys and per-op fixed costs
"""
import numpy as np
import ml_dtypes

import concourse.bass as bass
import concourse.bacc as bacc
import concourse.mybir as mybir
import concourse.tile as tile
from concourse.bass_utils import run_bass_kernel_spmd

F32 = mybir.dt.float32
BF16 = mybir.dt.bfloat16

L = 32
N = L ** 4          # 1048576
B = 32
NCORES = 8
BC = B // NCORES    # 4

_NC_CACHE = {}


def _build_nc():
    nc = bacc.Bacc("TRN2", target_bir_lowering=False, debug=False)

    # x pre-shuffled on host to [j0, (j1h, j2), (j1l, j3, b)], bf16
    x = nc.dram_tensor("x", [32, 131072], BF16, kind="ExternalInput")
    # boot block: augmented weights (w2|w3|w1|w0, 512 cols) followed by
    # the j0=0 input chunk (1024 cols) so one DMA covers the whole startup
    # critical path.
    wpack = nc.dram_tensor("wpack", [128, 1536], BF16, kind="ExternalInput")
    # y device order: [i2a(8), (i0, i3b)(128), (i3a, b, i1, i2b)(4096)] bf16
    y = nc.dram_tensor("y", [8, 524288], BF16, kind="ExternalOutput")

    def evac(eng, out_ap, in_ap):
        if eng == 'a':
            nc.scalar.copy(out=out_ap, in_=in_ap)
        else:
            nc.vector.tensor_copy(out_ap, in_ap)

    with tile.TileContext(nc) as tc:
        with tc.tile_pool(name="wp", bufs=1) as wp, \
             tc.tile_pool(name="zp", bufs=1) as zp, \
             tc.tile_pool(name="b1p", bufs=1) as b1p:
            ws = wp.tile([128, 1536], BF16, name="ws")
            nc.sync.dma_start(out=ws[:], in_=wpack.ap())
            boot_x = (ws.tensor, ws.offset + 512)
            w2s, w3s = ws[:, 0:128], ws[:, 128:256]
            w1s, w0s = ws[:, 256:384], ws[:, 384:512]

            # PE p-state warmup: zero matmuls ramp the tensor engine to
            # full clock before the first real matmul's data arrives.
            zt = zp.tile([128, 512], BF16, name="zt")
            nc.vector.memset(zt[:], 0)
            # B1: [part (i2b,j1), addr = i2a*4096 + i3a*512 + b*128 + i3b*32 + j0] bf16
            b1 = b1p.tile([128, 32768], BF16, name="b1")
            b1_t, b1_o = b1.tensor, b1.offset

            # ---- Phase I: S1 (contract j2) + S2 (contract j3), per j0 ----
            with tc.tile_pool(name="lp", bufs=4) as lp, \
                 tc.tile_pool(name="t1p", bufs=4) as t1p, \
                 tc.tile_pool(name="ps1", bufs=2, space="PSUM") as ps1, \
                 tc.tile_pool(name="ps2", bufs=2, space="PSUM") as ps2:
                p1_pre = ps1.tile([128, 1024], F32, name="p1")
                for d in range(2):
                    nc.tensor.matmul(p1_pre[:, 0:512], zt[:, 0:128], zt[:],
                                     start=True, stop=True)
                for j0 in range(32):
                    if j0 > 0:
                        lt = lp.tile([128, 1024], BF16, name="lt")
                        nc.sync.dma_start(
                            out=lt[:],
                            in_=bass.AP(x, j0 * 131072, [[1024, 128], [1, 1024]]))
                        halves = [(lt.tensor, lt.offset, 1024),
                                  (lt.tensor, lt.offset + 512, 1024)]
                    else:
                        # boot rows are 1536 wide; partition step must match
                        halves = [(boot_x[0], boot_x[1], 1536),
                                  (boot_x[0], boot_x[1] + 512, 1536)]

                    # T1: [part (j3, b), free addr = i2a*128+i2b*32+j1h*8+j1l]
                    t1 = t1p.tile([128, 1024], BF16, name="t1")
                    t1_t, t1_o = t1.tensor, t1.offset
                    p1 = p1_pre if j0 == 0 else ps1.tile([128, 1024], F32, name="p1")
                    for j1l in range(8):
                        ht, ho, hp = halves[j1l // 4]
                        lhsT = bass.AP(ht, ho + (j1l % 4) * 128,
                                       [[hp, 128], [1, 128]])
                        nc.tensor.matmul(p1[:, j1l * 128:(j1l + 1) * 128],
                                         lhsT, w2s, start=True, stop=True)
                    # psum pos (j1l, n1=(i2a,i2b,j1h)); merge (i2b,j1h)->[8,16]
                    # Fixed roles: chained S1 evacs on DVE, terminal S2 evacs
                    # on Act (measured optimal vs alternation under the
                    # warmed-up schedule).
                    e1, e2 = ('d', 'a')
                    evac(e1, bass.AP(t1_t, t1_o,
                                     [[1024, 128], [1, 8], [128, 8], [8, 16]]),
                         p1[:])

                    p2 = ps2.tile([128, 1024], F32, name="p2")
                    for i2a in range(8):
                        lhsT = bass.AP(t1_t, t1_o + i2a * 128,
                                       [[1024, 128], [1, 128]])
                        nc.tensor.matmul(p2[:, i2a * 128:(i2a + 1) * 128],
                                         lhsT, w3s, start=True, stop=True)
                    # psum pos (i2a, n2=(i3a,b,i3b)); merge (b,i3b)->[32,16]
                    if j0 == 31:
                        # Barrier split: phase II's first matmul group needs
                        # only b1's (k=0, i3a 0-1) region. Land that 32-col
                        # micro-piece first on the (idle) DVE, then the rest
                        # on Act, so stage 3 starts ~1us earlier.
                        evac('d', bass.AP(b1_t, b1_o + j0,
                                          [[32768, 128], [512, 2], [32, 16]]),
                             bass.AP(p2.tensor, p2.offset,
                                     [[1024, 128], [1, 32]]))
                        evac('a', bass.AP(b1_t, b1_o + j0 + 1024,
                                          [[32768, 128], [512, 6], [32, 16]]),
                             bass.AP(p2.tensor, p2.offset + 32,
                                     [[1024, 128], [1, 96]]))
                        evac('a', bass.AP(b1_t, b1_o + j0 + 4096,
                                          [[32768, 128], [4096, 7], [512, 8], [32, 16]]),
                             bass.AP(p2.tensor, p2.offset + 128,
                                     [[1024, 128], [1, 896]]))
                    else:
                        evac(e2, bass.AP(b1_t, b1_o + j0,
                                         [[32768, 128], [4096, 8], [512, 8], [32, 16]]),
                             p2[:])

            # ---- Phase II: S3 (contract j1) + S4 (contract j0), per i2a ----
            with tc.tile_pool(name="t3p", bufs=4) as t3p, \
                 tc.tile_pool(name="stgp", bufs=3) as stgp, \
                 tc.tile_pool(name="ps3", bufs=2, space="PSUM") as ps3, \
                 tc.tile_pool(name="ps4", bufs=2, space="PSUM") as ps4:
                for k in range(8):  # k = i2a
                    # T3: [part (i3b,j0), free (i3a:512, b:128, (i1*4+i2b):1)]
                    t3 = t3p.tile([128, 4096], BF16, name="t3")
                    t3_t, t3_o = t3.tensor, t3.offset
                    for th in range(4):  # pairs of i3a
                        p3 = ps3.tile([128, 1024], F32, name="p3")
                        for q in range(8):
                            cq = 8 * th + q      # cq = i3a*4 + b
                            lhsT = bass.AP(b1_t,
                                           b1_o + k * 4096 + cq * 128,
                                           [[32768, 128], [1, 128]])
                            nc.tensor.matmul(p3[:, q * 128:(q + 1) * 128],
                                             lhsT, w1s, start=True, stop=True)
                        # S3 evacs on DVE except three positions skewed to
                        # Act (empirically best load-balance points).
                        evac('a' if (4 * k + th) in (2, 10, 22) else 'd',
                             t3[:, th * 1024:(th + 1) * 1024], p3[:])

                    stg = stgp.tile([128, 4096], BF16, name="stg")
                    stg_t, stg_o = stg.tensor, stg.offset
                    for th in range(4):  # pairs of i3a
                        p4 = ps4.tile([128, 1024], F32, name="p4")
                        for m in range(2):
                            i3a = 2 * th + m
                            rhs = bass.AP(t3_t, t3_o + i3a * 512,
                                          [[4096, 128], [128, 4], [1, 128]])
                            nc.tensor.matmul(p4[:, m * 512:(m + 1) * 512],
                                             w0s, rhs, start=True, stop=True)
                        evac('a', stg[:, th * 1024:(th + 1) * 1024], p4[:])
                        nc.sync.dma_start(
                            out=bass.AP(y, k * 524288 + th * 1024,
                                        [[4096, 128], [1, 1024]]),
                            in_=bass.AP(stg_t, stg_o + th * 1024,
                                        [[4096, 128], [1, 1024]]))

    nc.finalize()
    return nc


def _build_waug(w: np.ndarray, kind: str) -> np.ndarray:
    """Augmented 128x128 weights (see baseline docstring)."""
    wa = np.zeros((128, 128), dtype=np.float32)
    ar = np.arange(32)
    if kind == "w3":
        # rows p = j3*4 + b ; cols n = i3a*16 + b*4 + i3b
        for b in range(4):
            cols = (ar >> 2) * 16 + b * 4 + (ar & 3)
            wa[np.ix_(ar * 4 + b, cols)] = w.T
    else:
        # rows p = q*32 + j ; cols n = i*4 + q
        for q in range(4):
            wa[np.ix_(q * 32 + ar, ar * 4 + q)] = w.T
    return wa


def _get_nc():
    if "nc" not in _NC_CACHE:
        _NC_CACHE["nc"] = _build_nc()
    return _NC_CACHE["nc"]


def make_in_maps(x, W0, W1, W2, W3):
    x = np.asarray(x, dtype=np.float32)
    bf = ml_dtypes.bfloat16
    wblock = np.concatenate([
        _build_waug(np.asarray(W2, np.float32), "q"),
        _build_waug(np.asarray(W3, np.float32), "w3"),
        _build_waug(np.asarray(W1, np.float32), "q"),
        _build_waug(np.asarray(W0, np.float32), "q"),
    ], axis=1).astype(bf)
    xr = x.reshape(32, 4, 8, 32, 32, B)
    in_maps = []
    for c in range(NCORES):
        xc = xr[..., c * BC:(c + 1) * BC].transpose(0, 1, 3, 2, 4, 5)
        xc = np.ascontiguousarray(xc).astype(bf).reshape(32, 131072)
        wpack = np.concatenate([wblock, xc[0].reshape(128, 1024)], axis=1)
        in_maps.append({"x": xc, "wpack": wpack})
    return in_maps


def _unshuffle_y(yd: np.ndarray) -> np.ndarray:
    """[i2a(8), (i0, i3b), (i3a, b, i1, i2b)] -> [N, BC]."""
    y = yd.astype(np.float32).reshape(8, 32, 4, 8, BC, 32, 4)
    y = y.transpose(1, 5, 0, 6, 3, 2, 4)
    return np.ascontiguousarray(y).reshape(N, BC)


def kernel(x, W0, W1, W2, W3, _trace=False):
    nc = _get_nc()
    in_maps = make_in_maps(x, W0, W1, W2, W3)
    res = run_bass_kernel_spmd(nc, in_maps, core_ids=list(range(NCORES)),
                               trace=_trace)
    out = np.concatenate(
        [_unshuffle_y(res.results[c]["y"]) for c in range(NCORES)], axis=1)
    if _trace:
        kernel.last_result = res
    return out


if __name__ == "__main__":
    rng = np.random.default_rng(0)
    x = rng.standard_normal((N, B), dtype=np.float32)
    ws = [rng.standard_normal((L, L), dtype=np.float32) for _ in range(4)]
    y = kernel(x, *ws)
    print("ran", y.shape, y.dtype)


# revision 11
# speedup vs baseline: 1.0154x; 1.0003x over previous
"""Trainium2 Bass kernel for (W0 (x) W1 (x) W2 (x) W3) @ x  -- Kronecker chain.

v3 over baseline (96167 -> 86876 ns in the TRN2 timeline model):
- boot DMA: augmented weights + the j0=0 input chunk in a single transfer,
  removing one DMA round-trip from the startup critical path
- per-th output stores (tail drains after the last 1024-col evac, not 4096)
- PE p-state warmup: two zero matmuls ramp the tensor engine to 2.4 GHz
  before the first input chunk lands
- evac engine assignment: fixed roles -- chained evacs (S1, S3) on DVE,
  terminal evacs (S2, S4) on Act -- with three S3 evacs skewed to Act at
  empirically chosen positions; all finer-grained skews/splits lose to
  scheduler convoys and per-op fixed costs
"""
import numpy as np
import ml_dtypes

import concourse.bass as bass
import concourse.bacc as bacc
import concourse.mybir as mybir
import concourse.tile as tile
from concourse.bass_utils import run_bass_kernel_spmd

F32 = mybir.dt.float32
BF16 = mybir.dt.bfloat16

L = 32
N = L ** 4          # 1048576
B = 32
NCORES = 8
BC = B // NCORES    # 4

_NC_CACHE = {}


def _build_nc():
    nc = bacc.Bacc("TRN2", target_bir_lowering=False, debug=False)

    # x pre-shuffled on host to [j0, (j1h, j2), (j1l, j3, b)], bf16
    x = nc.dram_tensor("x", [32, 131072], BF16, kind="ExternalInput")
    # boot block: augmented weights (w2|w3|w1|w0, 512 cols) followed by
    # the j0=0 input chunk (1024 cols) so one DMA covers the whole startup
    # critical path.
    wpack = nc.dram_tensor("wpack", [128, 1536], BF16, kind="ExternalInput")
    # y device order: [i2a(8), (i0, i3b)(128), (i3a, b, i1, i2b)(4096)] bf16
    y = nc.dram_tensor("y", [8, 524288], BF16, kind="ExternalOutput")

    def evac(eng, out_ap, in_ap):
        if eng == 'a':
            nc.scalar.copy(out=out_ap, in_=in_ap)
        else:
            nc.vector.tensor_copy(out_ap, in_ap)

    with tile.TileContext(nc) as tc:
        with tc.tile_pool(name="wp", bufs=1) as wp, \
             tc.tile_pool(name="zp", bufs=1) as zp, \
             tc.tile_pool(name="b1p", bufs=1) as b1p:
            # Boot in two DMAs: the first carries only what the first
            # matmul group needs (W2 block + first half of the j0=0 chunk),
            # the second the remaining weights + second half.
            wsa = wp.tile([128, 640], BF16, name="wsa")
            nc.sync.dma_start(out=wsa[:],
                              in_=bass.AP(wpack, 0, [[1536, 128], [1, 640]]))
            wsb = wp.tile([128, 896], BF16, name="wsb")
            nc.sync.dma_start(out=wsb[:],
                              in_=bass.AP(wpack, 640, [[1536, 128], [1, 896]]))
            w2s = wsa[:, 0:128]
            w3s, w1s, w0s = wsb[:, 0:128], wsb[:, 128:256], wsb[:, 256:384]

            # PE p-state warmup: zero matmuls ramp the tensor engine to
            # full clock before the first real matmul's data arrives.
            zt = zp.tile([128, 512], BF16, name="zt")
            nc.vector.memset(zt[:], 0)
            # B1: [part (i2b,j1), addr = i2a*4096 + i3a*512 + b*128 + i3b*32 + j0] bf16
            b1 = b1p.tile([128, 32768], BF16, name="b1")
            b1_t, b1_o = b1.tensor, b1.offset

            # ---- Phase I: S1 (contract j2) + S2 (contract j3), per j0 ----
            with tc.tile_pool(name="lp", bufs=4) as lp, \
                 tc.tile_pool(name="t1p", bufs=4) as t1p, \
                 tc.tile_pool(name="ps1", bufs=2, space="PSUM") as ps1, \
                 tc.tile_pool(name="ps2", bufs=2, space="PSUM") as ps2:
                p1_pre = ps1.tile([128, 1024], F32, name="p1")
                for d in range(2):
                    nc.tensor.matmul(p1_pre[:, 0:512], zt[:, 0:128], zt[:],
                                     start=True, stop=True)
                for j0 in range(32):
                    if j0 > 0:
                        lt = lp.tile([128, 1024], BF16, name="lt")
                        nc.sync.dma_start(
                            out=lt[:],
                            in_=bass.AP(x, j0 * 131072, [[1024, 128], [1, 1024]]))
                        halves = [(lt.tensor, lt.offset, 1024),
                                  (lt.tensor, lt.offset + 512, 1024)]
                    else:
                        # boot halves live in two tiles with their own pitches
                        halves = [(wsa.tensor, wsa.offset + 128, 640),
                                  (wsb.tensor, wsb.offset + 384, 896)]

                    # T1: [part (j3, b), free addr = i2a*128+i2b*32+j1h*8+j1l]
                    t1 = t1p.tile([128, 1024], BF16, name="t1")
                    t1_t, t1_o = t1.tensor, t1.offset
                    p1 = p1_pre if j0 == 0 else ps1.tile([128, 1024], F32, name="p1")
                    for j1l in range(8):
                        ht, ho, hp = halves[j1l // 4]
                        lhsT = bass.AP(ht, ho + (j1l % 4) * 128,
                                       [[hp, 128], [1, 128]])
                        nc.tensor.matmul(p1[:, j1l * 128:(j1l + 1) * 128],
                                         lhsT, w2s, start=True, stop=True)
                    # psum pos (j1l, n1=(i2a,i2b,j1h)); merge (i2b,j1h)->[8,16]
                    # Fixed roles: chained S1 evacs on DVE, terminal S2 evacs
                    # on Act (measured optimal vs alternation under the
                    # warmed-up schedule).
                    e1, e2 = ('d', 'a')
                    evac(e1, bass.AP(t1_t, t1_o,
                                     [[1024, 128], [1, 8], [128, 8], [8, 16]]),
                         p1[:])

                    p2 = ps2.tile([128, 1024], F32, name="p2")
                    for i2a in range(8):
                        lhsT = bass.AP(t1_t, t1_o + i2a * 128,
                                       [[1024, 128], [1, 128]])
                        nc.tensor.matmul(p2[:, i2a * 128:(i2a + 1) * 128],
                                         lhsT, w3s, start=True, stop=True)
                    # psum pos (i2a, n2=(i3a,b,i3b)); merge (b,i3b)->[32,16]
                    if j0 == 31:
                        # Barrier split: phase II's first matmul group needs
                        # only b1's (k=0, i3a 0-1) region. Land that 32-col
                        # micro-piece first on the (idle) DVE, then the rest
                        # on Act, so stage 3 starts ~1us earlier.
                        evac('d', bass.AP(b1_t, b1_o + j0,
                                          [[32768, 128], [512, 2], [32, 16]]),
                             bass.AP(p2.tensor, p2.offset,
                                     [[1024, 128], [1, 32]]))
                        evac('a', bass.AP(b1_t, b1_o + j0 + 1024,
                                          [[32768, 128], [512, 6], [32, 16]]),
                             bass.AP(p2.tensor, p2.offset + 32,
                                     [[1024, 128], [1, 96]]))
                        evac('a', bass.AP(b1_t, b1_o + j0 + 4096,
                                          [[32768, 128], [4096, 7], [512, 8], [32, 16]]),
                             bass.AP(p2.tensor, p2.offset + 128,
                                     [[1024, 128], [1, 896]]))
                    else:
                        evac(e2, bass.AP(b1_t, b1_o + j0,
                                         [[32768, 128], [4096, 8], [512, 8], [32, 16]]),
                             p2[:])

            # ---- Phase II: S3 (contract j1) + S4 (contract j0), per i2a ----
            with tc.tile_pool(name="t3p", bufs=4) as t3p, \
                 tc.tile_pool(name="stgp", bufs=3) as stgp, \
                 tc.tile_pool(name="ps3", bufs=2, space="PSUM") as ps3, \
                 tc.tile_pool(name="ps4", bufs=2, space="PSUM") as ps4:
                for k in range(8):  # k = i2a
                    # T3: [part (i3b,j0), free (i3a:512, b:128, (i1*4+i2b):1)]
                    t3 = t3p.tile([128, 4096], BF16, name="t3")
                    t3_t, t3_o = t3.tensor, t3.offset
                    for th in range(4):  # pairs of i3a
                        p3 = ps3.tile([128, 1024], F32, name="p3")
                        for q in range(8):
                            cq = 8 * th + q      # cq = i3a*4 + b
                            lhsT = bass.AP(b1_t,
                                           b1_o + k * 4096 + cq * 128,
                                           [[32768, 128], [1, 128]])
                            nc.tensor.matmul(p3[:, q * 128:(q + 1) * 128],
                                             lhsT, w1s, start=True, stop=True)
                        # S3 evacs on DVE except three positions skewed to
                        # Act (empirically best load-balance points).
                        evac('a' if (4 * k + th) in (2, 10, 22) else 'd',
                             t3[:, th * 1024:(th + 1) * 1024], p3[:])

                    stg = stgp.tile([128, 4096], BF16, name="stg")
                    stg_t, stg_o = stg.tensor, stg.offset
                    for th in range(4):  # pairs of i3a
                        p4 = ps4.tile([128, 1024], F32, name="p4")
                        for m in range(2):
                            i3a = 2 * th + m
                            rhs = bass.AP(t3_t, t3_o + i3a * 512,
                                          [[4096, 128], [128, 4], [1, 128]])
                            nc.tensor.matmul(p4[:, m * 512:(m + 1) * 512],
                                             w0s, rhs, start=True, stop=True)
                        evac('a', stg[:, th * 1024:(th + 1) * 1024], p4[:])
                        nc.sync.dma_start(
                            out=bass.AP(y, k * 524288 + th * 1024,
                                        [[4096, 128], [1, 1024]]),
                            in_=bass.AP(stg_t, stg_o + th * 1024,
                                        [[4096, 128], [1, 1024]]))

    nc.finalize()
    return nc


def _build_waug(w: np.ndarray, kind: str) -> np.ndarray:
    """Augmented 128x128 weights (see baseline docstring)."""
    wa = np.zeros((128, 128), dtype=np.float32)
    ar = np.arange(32)
    if kind == "w3":
        # rows p = j3*4 + b ; cols n = i3a*16 + b*4 + i3b
        for b in range(4):
            cols = (ar >> 2) * 16 + b * 4 + (ar & 3)
            wa[np.ix_(ar * 4 + b, cols)] = w.T
    else:
        # rows p = q*32 + j ; cols n = i*4 + q
        for q in range(4):
            wa[np.ix_(q * 32 + ar, ar * 4 + q)] = w.T
    return wa


def _get_nc():
    if "nc" not in _NC_CACHE:
        _NC_CACHE["nc"] = _build_nc()
    return _NC_CACHE["nc"]


def make_in_maps(x, W0, W1, W2, W3):
    x = np.asarray(x, dtype=np.float32)
    bf = ml_dtypes.bfloat16
    wblock = np.concatenate([
        _build_waug(np.asarray(W2, np.float32), "q"),
        _build_waug(np.asarray(W3, np.float32), "w3"),
        _build_waug(np.asarray(W1, np.float32), "q"),
        _build_waug(np.asarray(W0, np.float32), "q"),
    ], axis=1).astype(bf)
    xr = x.reshape(32, 4, 8, 32, 32, B)
    in_maps = []
    for c in range(NCORES):
        xc = xr[..., c * BC:(c + 1) * BC].transpose(0, 1, 3, 2, 4, 5)
        xc = np.ascontiguousarray(xc).astype(bf).reshape(32, 131072)
        x0 = xc[0].reshape(128, 1024)
        wpack = np.concatenate([wblock[:, 0:128], x0[:, 0:512],
                                wblock[:, 128:512], x0[:, 512:1024]], axis=1)
        in_maps.append({"x": xc, "wpack": wpack})
    return in_maps


def _unshuffle_y(yd: np.ndarray) -> np.ndarray:
    """[i2a(8), (i0, i3b), (i3a, b, i1, i2b)] -> [N, BC]."""
    y = yd.astype(np.float32).reshape(8, 32, 4, 8, BC, 32, 4)
    y = y.transpose(1, 5, 0, 6, 3, 2, 4)
    return np.ascontiguousarray(y).reshape(N, BC)


def kernel(x, W0, W1, W2, W3, _trace=False):
    nc = _get_nc()
    in_maps = make_in_maps(x, W0, W1, W2, W3)
    res = run_bass_kernel_spmd(nc, in_maps, core_ids=list(range(NCORES)),
                               trace=_trace)
    out = np.concatenate(
        [_unshuffle_y(res.results[c]["y"]) for c in range(NCORES)], axis=1)
    if _trace:
        kernel.last_result = res
    return out


if __name__ == "__main__":
    rng = np.random.default_rng(0)
    x = rng.standard_normal((N, B), dtype=np.float32)
    ws = [rng.standard_normal((L, L), dtype=np.float32) for _ in range(4)]
    y = kernel(x, *ws)
    print("ran", y.shape, y.dtype)


# revision 13
# speedup vs baseline: 1.0202x; 1.0047x over previous
"""Trainium2 Bass kernel for (W0 (x) W1 (x) W2 (x) W3) @ x  -- Kronecker chain.

v3 over baseline (96167 -> 86447 ns in the TRN2 timeline model):
- boot DMAs: augmented weights + the j0=0 input chunk ship in two transfers
  split along the startup critical path (W2 + first half-chunk first)
- per-th output stores (tail drains after the last 1024-col evac, not 4096)
- PE p-state warmup: two zero matmuls ramp the tensor engine to 2.4 GHz
  before the first input chunk lands
- evac engine assignment: fixed roles -- chained evacs (S1, S3) on DVE,
  terminal evacs (S2, S4) on Act -- with three S3 evacs skewed to Act at
  empirically chosen positions; all finer-grained skews/splits lose to
  scheduler convoys and per-op fixed costs
"""
import numpy as np
import ml_dtypes

import concourse.bass as bass
import concourse.bacc as bacc
import concourse.mybir as mybir
import concourse.tile as tile
from concourse.bass_utils import run_bass_kernel_spmd

F32 = mybir.dt.float32
BF16 = mybir.dt.bfloat16

L = 32
N = L ** 4          # 1048576
B = 32
NCORES = 8
BC = B // NCORES    # 4

_NC_CACHE = {}


def _build_nc():
    nc = bacc.Bacc("TRN2", target_bir_lowering=False, debug=False)

    # x pre-shuffled on host to [j0, (j1h, j2), (j1l, j3, b)], bf16
    x = nc.dram_tensor("x", [32, 131072], BF16, kind="ExternalInput")
    # boot block, host layout: [w2 | x0 first half | w3 w1 w0 | x0 second
    # half] so two DMAs cover the startup critical path in consumer order.
    wpack = nc.dram_tensor("wpack", [128, 1536], BF16, kind="ExternalInput")
    # y device order: [i2a(8), (i0, i3b)(128), (i3a, b, i1, i2b)(4096)] bf16
    y = nc.dram_tensor("y", [8, 524288], BF16, kind="ExternalOutput")

    def evac(eng, out_ap, in_ap):
        if eng == 'a':
            nc.scalar.copy(out=out_ap, in_=in_ap)
        else:
            nc.vector.tensor_copy(out_ap, in_ap)

    with tile.TileContext(nc) as tc:
        with tc.tile_pool(name="wp", bufs=1) as wp, \
             tc.tile_pool(name="zp", bufs=1) as zp, \
             tc.tile_pool(name="b1p", bufs=1) as b1p:
            # Boot in two DMAs: the first carries only what the first
            # matmul group needs (W2 block + first half of the j0=0 chunk),
            # the second the remaining weights + second half.
            wsa = wp.tile([128, 640], BF16, name="wsa")
            nc.sync.dma_start(out=wsa[:],
                              in_=bass.AP(wpack, 0, [[1536, 128], [1, 640]]))
            wsb = wp.tile([128, 896], BF16, name="wsb")
            nc.sync.dma_start(out=wsb[:],
                              in_=bass.AP(wpack, 640, [[1536, 128], [1, 896]]))
            w2s = wsa[:, 0:128]
            w3s, w1s, w0s = wsb[:, 0:128], wsb[:, 128:256], wsb[:, 256:384]

            # PE p-state warmup: zero matmuls ramp the tensor engine to
            # full clock before the first real matmul's data arrives.
            zt = zp.tile([128, 512], BF16, name="zt")
            nc.vector.memset(zt[:], 0)
            # B1: [part (i2b,j1), addr = i2a*4096 + i3a*512 + b*128 + i3b*32 + j0] bf16
            b1 = b1p.tile([128, 32768], BF16, name="b1")
            b1_t, b1_o = b1.tensor, b1.offset

            # ---- Phase I: S1 (contract j2) + S2 (contract j3), per j0 ----
            with tc.tile_pool(name="lp", bufs=4) as lp, \
                 tc.tile_pool(name="t1p", bufs=4) as t1p, \
                 tc.tile_pool(name="ps1", bufs=2, space="PSUM") as ps1, \
                 tc.tile_pool(name="ps2", bufs=2, space="PSUM") as ps2:
                p1_pre = ps1.tile([128, 1024], F32, name="p1")
                for d in range(2):
                    nc.tensor.matmul(p1_pre[:, 0:512], zt[:, 0:128], zt[:],
                                     start=True, stop=True)
                for j0 in range(32):
                    if j0 > 0:
                        lt = lp.tile([128, 1024], BF16, name="lt")
                        nc.sync.dma_start(
                            out=lt[:],
                            in_=bass.AP(x, j0 * 131072, [[1024, 128], [1, 1024]]))
                        halves = [(lt.tensor, lt.offset, 1024),
                                  (lt.tensor, lt.offset + 512, 1024)]
                    else:
                        # boot halves live in two tiles with their own pitches
                        halves = [(wsa.tensor, wsa.offset + 128, 640),
                                  (wsb.tensor, wsb.offset + 384, 896)]

                    # T1: [part (j3, b), free addr = i2a*128+i2b*32+j1h*8+j1l]
                    t1 = t1p.tile([128, 1024], BF16, name="t1")
                    t1_t, t1_o = t1.tensor, t1.offset
                    p1 = p1_pre if j0 == 0 else ps1.tile([128, 1024], F32, name="p1")
                    for j1l in range(8):
                        ht, ho, hp = halves[j1l // 4]
                        lhsT = bass.AP(ht, ho + (j1l % 4) * 128,
                                       [[hp, 128], [1, 128]])
                        nc.tensor.matmul(p1[:, j1l * 128:(j1l + 1) * 128],
                                         lhsT, w2s, start=True, stop=True)
                    # psum pos (j1l, n1=(i2a,i2b,j1h)); merge (i2b,j1h)->[8,16]
                    # Fixed roles: chained S1 evacs on DVE, terminal S2 evacs
                    # on Act (measured optimal vs alternation under the
                    # warmed-up schedule).
                    e1, e2 = ('d', 'a')
                    evac(e1, bass.AP(t1_t, t1_o,
                                     [[1024, 128], [1, 8], [128, 8], [8, 16]]),
                         p1[:])

                    p2 = ps2.tile([128, 1024], F32, name="p2")
                    for i2a in range(8):
                        lhsT = bass.AP(t1_t, t1_o + i2a * 128,
                                       [[1024, 128], [1, 128]])
                        nc.tensor.matmul(p2[:, i2a * 128:(i2a + 1) * 128],
                                         lhsT, w3s, start=True, stop=True)
                    # psum pos (i2a, n2=(i3a,b,i3b)); merge (b,i3b)->[32,16]
                    if j0 == 31:
                        # Barrier split: phase II's first matmul group needs
                        # only b1's (k=0, i3a 0-1) region. Land that 32-col
                        # micro-piece first on the (idle) DVE, then the rest
                        # on Act, so stage 3 starts ~1us earlier.
                        evac('d', bass.AP(b1_t, b1_o + j0,
                                          [[32768, 128], [512, 2], [32, 16]]),
                             bass.AP(p2.tensor, p2.offset,
                                     [[1024, 128], [1, 32]]))
                        evac('a', bass.AP(b1_t, b1_o + j0 + 1024,
                                          [[32768, 128], [512, 6], [32, 16]]),
                             bass.AP(p2.tensor, p2.offset + 32,
                                     [[1024, 128], [1, 96]]))
                        evac('a', bass.AP(b1_t, b1_o + j0 + 4096,
                                          [[32768, 128], [4096, 7], [512, 8], [32, 16]]),
                             bass.AP(p2.tensor, p2.offset + 128,
                                     [[1024, 128], [1, 896]]))
                    else:
                        evac(e2, bass.AP(b1_t, b1_o + j0,
                                         [[32768, 128], [4096, 8], [512, 8], [32, 16]]),
                             p2[:])

            # ---- Phase II: S3 (contract j1) + S4 (contract j0), per i2a ----
            with tc.tile_pool(name="t3p", bufs=4) as t3p, \
                 tc.tile_pool(name="stgp", bufs=3) as stgp, \
                 tc.tile_pool(name="ps3", bufs=2, space="PSUM") as ps3, \
                 tc.tile_pool(name="ps4", bufs=2, space="PSUM") as ps4:
                for k in range(8):  # k = i2a
                    # T3: [part (i3b,j0), free (i3a:512, b:128, (i1*4+i2b):1)]
                    t3 = t3p.tile([128, 4096], BF16, name="t3")
                    t3_t, t3_o = t3.tensor, t3.offset
                    for th in range(4):  # pairs of i3a
                        # At the three Act-skewed positions, borrow the psum
                        # tile from the ps4 pool: the late-returning Act evac
                        # then blocks ps4's rotation (slack there) instead of
                        # stalling the DVE-paced S3 stream via ps3 reuse.
                        if (4 * k + th) in (2, 10, 22):
                            p3 = ps4.tile([128, 1024], F32, name="p4")
                        else:
                            p3 = ps3.tile([128, 1024], F32, name="p3")
                        for q in range(8):
                            cq = 8 * th + q      # cq = i3a*4 + b
                            lhsT = bass.AP(b1_t,
                                           b1_o + k * 4096 + cq * 128,
                                           [[32768, 128], [1, 128]])
                            nc.tensor.matmul(p3[:, q * 128:(q + 1) * 128],
                                             lhsT, w1s, start=True, stop=True)
                        # S3 evacs on DVE except three positions skewed to
                        # Act (empirically best load-balance points).
                        evac('a' if (4 * k + th) in (2, 10, 22) else 'd',
                             t3[:, th * 1024:(th + 1) * 1024], p3[:])

                    stg = stgp.tile([128, 4096], BF16, name="stg")
                    stg_t, stg_o = stg.tensor, stg.offset
                    for th in range(4):  # pairs of i3a
                        p4 = ps4.tile([128, 1024], F32, name="p4")
                        for m in range(2):
                            i3a = 2 * th + m
                            rhs = bass.AP(t3_t, t3_o + i3a * 512,
                                          [[4096, 128], [128, 4], [1, 128]])
                            nc.tensor.matmul(p4[:, m * 512:(m + 1) * 512],
                                             w0s, rhs, start=True, stop=True)
                        evac('a', stg[:, th * 1024:(th + 1) * 1024], p4[:])
                        nc.sync.dma_start(
                            out=bass.AP(y, k * 524288 + th * 1024,
                                        [[4096, 128], [1, 1024]]),
                            in_=bass.AP(stg_t, stg_o + th * 1024,
                                        [[4096, 128], [1, 1024]]))

    nc.finalize()
    return nc


def _build_waug(w: np.ndarray, kind: str) -> np.ndarray:
    """Augmented 128x128 weights (see baseline docstring)."""
    wa = np.zeros((128, 128), dtype=np.float32)
    ar = np.arange(32)
    if kind == "w3":
        # rows p = j3*4 + b ; cols n = i3a*16 + b*4 + i3b
        for b in range(4):
            cols = (ar >> 2) * 16 + b * 4 + (ar & 3)
            wa[np.ix_(ar * 4 + b, cols)] = w.T
    else:
        # rows p = q*32 + j ; cols n = i*4 + q
        for q in range(4):
            wa[np.ix_(q * 32 + ar, ar * 4 + q)] = w.T
    return wa


def _get_nc():
    if "nc" not in _NC_CACHE:
        _NC_CACHE["nc"] = _build_nc()
    return _NC_CACHE["nc"]


def make_in_maps(x, W0, W1, W2, W3):
    x = np.asarray(x, dtype=np.float32)
    bf = ml_dtypes.bfloat16
    wblock = np.concatenate([
        _build_waug(np.asarray(W2, np.float32), "q"),
        _build_waug(np.asarray(W3, np.float32), "w3"),
        _build_waug(np.asarray(W1, np.float32), "q"),
        _build_waug(np.asarray(W0, np.float32), "q"),
    ], axis=1).astype(bf)
    xr = x.reshape(32, 4, 8, 32, 32, B)
    in_maps = []
    for c in range(NCORES):
        xc = xr[..., c * BC:(c + 1) * BC].transpose(0, 1, 3, 2, 4, 5)
        xc = np.ascontiguousarray(xc).astype(bf).reshape(32, 131072)
        x0 = xc[0].reshape(128, 1024)
        wpack = np.concatenate([wblock[:, 0:128], x0[:, 0:512],
                                wblock[:, 128:512], x0[:, 512:1024]], axis=1)
        in_maps.append({"x": xc, "wpack": wpack})
    return in_maps


def _unshuffle_y(yd: np.ndarray) -> np.ndarray:
    """[i2a(8), (i0, i3b), (i3a, b, i1, i2b)] -> [N, BC]."""
    y = yd.astype(np.float32).reshape(8, 32, 4, 8, BC, 32, 4)
    y = y.transpose(1, 5, 0, 6, 3, 2, 4)
    return np.ascontiguousarray(y).reshape(N, BC)


def kernel(x, W0, W1, W2, W3, _trace=False):
    nc = _get_nc()
    in_maps = make_in_maps(x, W0, W1, W2, W3)
    res = run_bass_kernel_spmd(nc, in_maps, core_ids=list(range(NCORES)),
                               trace=_trace)
    out = np.concatenate(
        [_unshuffle_y(res.results[c]["y"]) for c in range(NCORES)], axis=1)
    if _trace:
        kernel.last_result = res
    return out


if __name__ == "__main__":
    rng = np.random.default_rng(0)
    x = rng.standard_normal((N, B), dtype=np.float32)
    ws = [rng.standard_normal((L, L), dtype=np.float32) for _ in range(4)]
    y = kernel(x, *ws)
    print("ran", y.shape, y.dtype)


# revision 14
# speedup vs baseline: 1.0241x; 1.0039x over previous
"""Trainium2 Bass kernel for (W0 (x) W1 (x) W2 (x) W3) @ x  -- Kronecker chain.

v3 over baseline (96167 -> 86114 ns in the TRN2 timeline model):
- boot DMAs: augmented weights + the j0=0 input chunk ship in two transfers
  split along the startup critical path (W2 + first half-chunk first)
- per-th output stores (tail drains after the last 1024-col evac, not 4096)
- PE p-state warmup: two zero matmuls ramp the tensor engine to 2.4 GHz
  before the first input chunk lands
- evac engine assignment: fixed roles -- chained evacs (S1, S3) on DVE,
  terminal evacs (S2, S4) on Act -- with three S3 evacs skewed to Act at
  empirically chosen positions; all finer-grained skews/splits lose to
  scheduler convoys and per-op fixed costs
"""
import numpy as np
import ml_dtypes

import concourse.bass as bass
import concourse.bacc as bacc
import concourse.mybir as mybir
import concourse.tile as tile
from concourse.bass_utils import run_bass_kernel_spmd

F32 = mybir.dt.float32
BF16 = mybir.dt.bfloat16

L = 32
N = L ** 4          # 1048576
B = 32
NCORES = 8
BC = B // NCORES    # 4

_NC_CACHE = {}


def _build_nc():
    nc = bacc.Bacc("TRN2", target_bir_lowering=False, debug=False)

    # x pre-shuffled on host to [j0, (j1h, j2), (j1l, j3, b)], bf16
    x = nc.dram_tensor("x", [32, 131072], BF16, kind="ExternalInput")
    # boot block, host layout: [w2 | x0 first half | w3 w1 w0 | x0 second
    # half] so two DMAs cover the startup critical path in consumer order.
    wpack = nc.dram_tensor("wpack", [128, 1536], BF16, kind="ExternalInput")
    # y device order: [i2a(8), (i0, i3b)(128), (i3a, b, i1, i2b)(4096)] bf16
    y = nc.dram_tensor("y", [8, 524288], BF16, kind="ExternalOutput")

    def evac(eng, out_ap, in_ap):
        if eng == 'a':
            nc.scalar.copy(out=out_ap, in_=in_ap)
        else:
            nc.vector.tensor_copy(out_ap, in_ap)

    with tile.TileContext(nc) as tc:
        with tc.tile_pool(name="wp", bufs=1) as wp, \
             tc.tile_pool(name="zp", bufs=1) as zp, \
             tc.tile_pool(name="b1p", bufs=1) as b1p:
            # Boot in two DMAs: the first carries only what the first
            # matmul group needs (W2 block + first half of the j0=0 chunk),
            # the second the remaining weights + second half.
            wsa = wp.tile([128, 640], BF16, name="wsa")
            nc.sync.dma_start(out=wsa[:],
                              in_=bass.AP(wpack, 0, [[1536, 128], [1, 640]]))
            wsb = wp.tile([128, 896], BF16, name="wsb")
            nc.sync.dma_start(out=wsb[:],
                              in_=bass.AP(wpack, 640, [[1536, 128], [1, 896]]))
            w2s = wsa[:, 0:128]
            w3s, w1s, w0s = wsb[:, 0:128], wsb[:, 128:256], wsb[:, 256:384]

            # PE p-state warmup: zero matmuls ramp the tensor engine to
            # full clock before the first real matmul's data arrives.
            zt = zp.tile([128, 512], BF16, name="zt")
            nc.vector.memset(zt[:], 0)
            # B1: [part (i2b,j1), addr = i2a*4096 + i3a*512 + b*128 + i3b*32 + j0] bf16
            b1 = b1p.tile([128, 32768], BF16, name="b1")
            b1_t, b1_o = b1.tensor, b1.offset

            # ---- Phase I: S1 (contract j2) + S2 (contract j3), per j0 ----
            with tc.tile_pool(name="lp", bufs=4) as lp, \
                 tc.tile_pool(name="t1p", bufs=4) as t1p, \
                 tc.tile_pool(name="ps1", bufs=2, space="PSUM") as ps1, \
                 tc.tile_pool(name="ps2", bufs=2, space="PSUM") as ps2:
                p1_pre = ps1.tile([128, 1024], F32, name="p1")
                for d in range(2):
                    nc.tensor.matmul(p1_pre[:, 0:512], zt[:, 0:128], zt[:],
                                     start=True, stop=True)
                for j0 in range(32):
                    if j0 > 0:
                        lt = lp.tile([128, 1024], BF16, name="lt")
                        nc.sync.dma_start(
                            out=lt[:],
                            in_=bass.AP(x, j0 * 131072, [[1024, 128], [1, 1024]]))
                        halves = [(lt.tensor, lt.offset, 1024),
                                  (lt.tensor, lt.offset + 512, 1024)]
                    else:
                        # boot halves live in two tiles with their own pitches
                        halves = [(wsa.tensor, wsa.offset + 128, 640),
                                  (wsb.tensor, wsb.offset + 384, 896)]

                    # T1: [part (j3, b), free addr = i2a*128+i2b*32+j1h*8+j1l]
                    t1 = t1p.tile([128, 1024], BF16, name="t1")
                    t1_t, t1_o = t1.tensor, t1.offset
                    p1 = p1_pre if j0 == 0 else ps1.tile([128, 1024], F32, name="p1")
                    for j1l in range(8):
                        ht, ho, hp = halves[j1l // 4]
                        lhsT = bass.AP(ht, ho + (j1l % 4) * 128,
                                       [[hp, 128], [1, 128]])
                        nc.tensor.matmul(p1[:, j1l * 128:(j1l + 1) * 128],
                                         lhsT, w2s, start=True, stop=True)
                    # psum pos (j1l, n1=(i2a,i2b,j1h)); merge (i2b,j1h)->[8,16]
                    # Fixed roles: chained S1 evacs on DVE, terminal S2 evacs
                    # on Act (measured optimal vs alternation under the
                    # warmed-up schedule).
                    e1, e2 = ('d', 'a')
                    evac(e1, bass.AP(t1_t, t1_o,
                                     [[1024, 128], [1, 8], [128, 8], [8, 16]]),
                         p1[:])

                    p2 = ps2.tile([128, 1024], F32, name="p2")
                    for i2a in range(8):
                        lhsT = bass.AP(t1_t, t1_o + i2a * 128,
                                       [[1024, 128], [1, 128]])
                        nc.tensor.matmul(p2[:, i2a * 128:(i2a + 1) * 128],
                                         lhsT, w3s, start=True, stop=True)
                    # psum pos (i2a, n2=(i3a,b,i3b)); merge (b,i3b)->[32,16]
                    if j0 == 31:
                        # Barrier split: phase II's first matmul group needs
                        # only b1's (k=0, i3a 0-1) region. Land that 32-col
                        # micro-piece first on the (idle) DVE, then the rest
                        # on Act, so stage 3 starts ~1us earlier.
                        evac('d', bass.AP(b1_t, b1_o + j0,
                                          [[32768, 128], [512, 2], [32, 16]]),
                             bass.AP(p2.tensor, p2.offset,
                                     [[1024, 128], [1, 32]]))
                        evac('a', bass.AP(b1_t, b1_o + j0 + 1024,
                                          [[32768, 128], [512, 6], [32, 16]]),
                             bass.AP(p2.tensor, p2.offset + 32,
                                     [[1024, 128], [1, 96]]))
                        evac('a', bass.AP(b1_t, b1_o + j0 + 4096,
                                          [[32768, 128], [4096, 7], [512, 8], [32, 16]]),
                             bass.AP(p2.tensor, p2.offset + 128,
                                     [[1024, 128], [1, 896]]))
                    else:
                        evac(e2, bass.AP(b1_t, b1_o + j0,
                                         [[32768, 128], [4096, 8], [512, 8], [32, 16]]),
                             p2[:])

            # ---- Phase II: S3 (contract j1) + S4 (contract j0), per i2a ----
            with tc.tile_pool(name="t3p", bufs=4) as t3p, \
                 tc.tile_pool(name="stgp", bufs=3) as stgp, \
                 tc.tile_pool(name="ps3", bufs=2, space="PSUM") as ps3, \
                 tc.tile_pool(name="ps4", bufs=2, space="PSUM") as ps4:
                for k in range(8):  # k = i2a
                    # T3: [part (i3b,j0), free (i3a:512, b:128, (i1*4+i2b):1)]
                    t3 = t3p.tile([128, 4096], BF16, name="t3")
                    t3_t, t3_o = t3.tensor, t3.offset
                    for th in range(4):  # pairs of i3a
                        # At the three Act-skewed positions, borrow the psum
                        # tile from the ps4 pool: the late-returning Act evac
                        # then blocks ps4's rotation (slack there) instead of
                        # stalling the DVE-paced S3 stream via ps3 reuse.
                        if (4 * k + th) in (2, 10, 22):
                            p3 = ps4.tile([128, 1024], F32, name="p4")
                        else:
                            p3 = ps3.tile([128, 1024], F32, name="p3")
                        for q in range(8):
                            cq = 8 * th + q      # cq = i3a*4 + b
                            lhsT = bass.AP(b1_t,
                                           b1_o + k * 4096 + cq * 128,
                                           [[32768, 128], [1, 128]])
                            nc.tensor.matmul(p3[:, q * 128:(q + 1) * 128],
                                             lhsT, w1s, start=True, stop=True)
                        # S3 evacs on DVE except three positions skewed to
                        # Act (empirically best load-balance points).
                        evac('a' if (4 * k + th) in (2, 10, 22) else 'd',
                             t3[:, th * 1024:(th + 1) * 1024], p3[:])

                    stg = stgp.tile([128, 4096], BF16, name="stg")
                    stg_t, stg_o = stg.tensor, stg.offset
                    for th in range(4):  # pairs of i3a
                        p4 = ps4.tile([128, 1024], F32, name="p4")
                        for m in range(2):
                            i3a = 2 * th + m
                            rhs = bass.AP(t3_t, t3_o + i3a * 512,
                                          [[4096, 128], [128, 4], [1, 128]])
                            nc.tensor.matmul(p4[:, m * 512:(m + 1) * 512],
                                             w0s, rhs, start=True, stop=True)
                        # S4 evacs on Act, except (7,2) handed to DVE:
                        # DVE finishes its S3 stream ~3us before Act drains
                        # its k=7 backlog, so it absorbs one late unit.
                        evac('d' if (k, th) == (7, 2) else 'a',
                             stg[:, th * 1024:(th + 1) * 1024], p4[:])
                        nc.sync.dma_start(
                            out=bass.AP(y, k * 524288 + th * 1024,
                                        [[4096, 128], [1, 1024]]),
                            in_=bass.AP(stg_t, stg_o + th * 1024,
                                        [[4096, 128], [1, 1024]]))

    nc.finalize()
    return nc


def _build_waug(w: np.ndarray, kind: str) -> np.ndarray:
    """Augmented 128x128 weights (see baseline docstring)."""
    wa = np.zeros((128, 128), dtype=np.float32)
    ar = np.arange(32)
    if kind == "w3":
        # rows p = j3*4 + b ; cols n = i3a*16 + b*4 + i3b
        for b in range(4):
            cols = (ar >> 2) * 16 + b * 4 + (ar & 3)
            wa[np.ix_(ar * 4 + b, cols)] = w.T
    else:
        # rows p = q*32 + j ; cols n = i*4 + q
        for q in range(4):
            wa[np.ix_(q * 32 + ar, ar * 4 + q)] = w.T
    return wa


def _get_nc():
    if "nc" not in _NC_CACHE:
        _NC_CACHE["nc"] = _build_nc()
    return _NC_CACHE["nc"]


def make_in_maps(x, W0, W1, W2, W3):
    x = np.asarray(x, dtype=np.float32)
    bf = ml_dtypes.bfloat16
    wblock = np.concatenate([
        _build_waug(np.asarray(W2, np.float32), "q"),
        _build_waug(np.asarray(W3, np.float32), "w3"),
        _build_waug(np.asarray(W1, np.float32), "q"),
        _build_waug(np.asarray(W0, np.float32), "q"),
    ], axis=1).astype(bf)
    xr = x.reshape(32, 4, 8, 32, 32, B)
    in_maps = []
    for c in range(NCORES):
        xc = xr[..., c * BC:(c + 1) * BC].transpose(0, 1, 3, 2, 4, 5)
        xc = np.ascontiguousarray(xc).astype(bf).reshape(32, 131072)
        x0 = xc[0].reshape(128, 1024)
        wpack = np.concatenate([wblock[:, 0:128], x0[:, 0:512],
                                wblock[:, 128:512], x0[:, 512:1024]], axis=1)
        in_maps.append({"x": xc, "wpack": wpack})
    return in_maps


def _unshuffle_y(yd: np.ndarray) -> np.ndarray:
    """[i2a(8), (i0, i3b), (i3a, b, i1, i2b)] -> [N, BC]."""
    y = yd.astype(np.float32).reshape(8, 32, 4, 8, BC, 32, 4)
    y = y.transpose(1, 5, 0, 6, 3, 2, 4)
    return np.ascontiguousarray(y).reshape(N, BC)


def kernel(x, W0, W1, W2, W3, _trace=False):
    nc = _get_nc()
    in_maps = make_in_maps(x, W0, W1, W2, W3)
    res = run_bass_kernel_spmd(nc, in_maps, core_ids=list(range(NCORES)),
                               trace=_trace)
    out = np.concatenate(
        [_unshuffle_y(res.results[c]["y"]) for c in range(NCORES)], axis=1)
    if _trace:
        kernel.last_result = res
    return out


if __name__ == "__main__":
    rng = np.random.default_rng(0)
    x = rng.standard_normal((N, B), dtype=np.float32)
    ws = [rng.standard_normal((L, L), dtype=np.float32) for _ in range(4)]
    y = kernel(x, *ws)
    print("ran", y.shape, y.dtype)


# revision 15
# speedup vs baseline: 1.0253x; 1.0012x over previous
"""Trainium2 Bass kernel for (W0 (x) W1 (x) W2 (x) W3) @ x  -- Kronecker chain.

v3 over baseline (96167 -> 86015 ns in the TRN2 timeline model):
- boot DMAs: augmented weights + the j0=0 input chunk ship in two transfers
  split along the startup critical path (W2 + first half-chunk first)
- per-th output stores (tail drains after the last 1024-col evac, not 4096)
- PE p-state warmup: two zero matmuls ramp the tensor engine to 2.4 GHz
  before the first input chunk lands
- evac engine assignment: fixed roles -- chained evacs (S1, S3) on DVE,
  terminal evacs (S2, S4) on Act -- with three S3 evacs skewed to Act at
  empirically chosen positions; all finer-grained skews/splits lose to
  scheduler convoys and per-op fixed costs
"""
import numpy as np
import ml_dtypes

import concourse.bass as bass
import concourse.bacc as bacc
import concourse.mybir as mybir
import concourse.tile as tile
from concourse.bass_utils import run_bass_kernel_spmd

F32 = mybir.dt.float32
BF16 = mybir.dt.bfloat16

L = 32
N = L ** 4          # 1048576
B = 32
NCORES = 8
BC = B // NCORES    # 4

_NC_CACHE = {}


def _build_nc():
    nc = bacc.Bacc("TRN2", target_bir_lowering=False, debug=False)

    # x pre-shuffled on host to [j0, (j1h, j2), (j1l, j3, b)], bf16
    x = nc.dram_tensor("x", [32, 131072], BF16, kind="ExternalInput")
    # boot block, host layout: [w2 | x0 first half | w3 w1 w0 | x0 second
    # half] so two DMAs cover the startup critical path in consumer order.
    wpack = nc.dram_tensor("wpack", [128, 1536], BF16, kind="ExternalInput")
    # y device order: [i2a(8), (i0, i3b)(128), (i3a, b, i1, i2b)(4096)] bf16
    y = nc.dram_tensor("y", [8, 524288], BF16, kind="ExternalOutput")

    def evac(eng, out_ap, in_ap):
        if eng == 'a':
            nc.scalar.copy(out=out_ap, in_=in_ap)
        else:
            nc.vector.tensor_copy(out_ap, in_ap)

    with tile.TileContext(nc) as tc:
        with tc.tile_pool(name="wp", bufs=1) as wp, \
             tc.tile_pool(name="zp", bufs=1) as zp, \
             tc.tile_pool(name="b1p", bufs=1) as b1p:
            # Boot in two DMAs: the first carries only what the first
            # matmul group needs (W2 block + first half of the j0=0 chunk),
            # the second the remaining weights + second half.
            wsa = wp.tile([128, 640], BF16, name="wsa")
            nc.sync.dma_start(out=wsa[:],
                              in_=bass.AP(wpack, 0, [[1536, 128], [1, 640]]))
            wsb = wp.tile([128, 896], BF16, name="wsb")
            nc.sync.dma_start(out=wsb[:],
                              in_=bass.AP(wpack, 640, [[1536, 128], [1, 896]]))
            w2s = wsa[:, 0:128]
            w3s, w1s, w0s = wsb[:, 0:128], wsb[:, 128:256], wsb[:, 256:384]

            # PE p-state warmup: zero matmuls ramp the tensor engine to
            # full clock before the first real matmul's data arrives.
            zt = zp.tile([128, 512], BF16, name="zt")
            nc.vector.memset(zt[:], 0)
            # B1: [part (i2b,j1), addr = i2a*4096 + i3a*512 + b*128 + i3b*32 + j0] bf16
            b1 = b1p.tile([128, 32768], BF16, name="b1")
            b1_t, b1_o = b1.tensor, b1.offset

            # ---- Phase I: S1 (contract j2) + S2 (contract j3), per j0 ----
            with tc.tile_pool(name="lp", bufs=4) as lp, \
                 tc.tile_pool(name="t1p", bufs=4) as t1p, \
                 tc.tile_pool(name="ps1", bufs=2, space="PSUM") as ps1, \
                 tc.tile_pool(name="ps2", bufs=2, space="PSUM") as ps2:
                p1_pre = ps1.tile([128, 1024], F32, name="p1")
                for d in range(2):
                    nc.tensor.matmul(p1_pre[:, 0:512], zt[:, 0:128], zt[:],
                                     start=True, stop=True)
                for j0 in range(32):
                    if j0 > 0:
                        lt = lp.tile([128, 1024], BF16, name="lt")
                        nc.sync.dma_start(
                            out=lt[:],
                            in_=bass.AP(x, j0 * 131072, [[1024, 128], [1, 1024]]))
                        halves = [(lt.tensor, lt.offset, 1024),
                                  (lt.tensor, lt.offset + 512, 1024)]
                    else:
                        # boot halves live in two tiles with their own pitches
                        halves = [(wsa.tensor, wsa.offset + 128, 640),
                                  (wsb.tensor, wsb.offset + 384, 896)]

                    # T1: [part (j3, b), free addr = i2a*128+i2b*32+j1h*8+j1l]
                    t1 = t1p.tile([128, 1024], BF16, name="t1")
                    t1_t, t1_o = t1.tensor, t1.offset
                    p1 = p1_pre if j0 == 0 else ps1.tile([128, 1024], F32, name="p1")
                    for j1l in range(8):
                        ht, ho, hp = halves[j1l // 4]
                        lhsT = bass.AP(ht, ho + (j1l % 4) * 128,
                                       [[hp, 128], [1, 128]])
                        nc.tensor.matmul(p1[:, j1l * 128:(j1l + 1) * 128],
                                         lhsT, w2s, start=True, stop=True)
                    # psum pos (j1l, n1=(i2a,i2b,j1h)); merge (i2b,j1h)->[8,16]
                    # Fixed roles: chained S1 evacs on DVE, terminal S2 evacs
                    # on Act (measured optimal vs alternation under the
                    # warmed-up schedule).
                    e1, e2 = ('d', 'a')
                    evac(e1, bass.AP(t1_t, t1_o,
                                     [[1024, 128], [1, 8], [128, 8], [8, 16]]),
                         p1[:])

                    p2 = ps2.tile([128, 1024], F32, name="p2")
                    for i2a in range(8):
                        lhsT = bass.AP(t1_t, t1_o + i2a * 128,
                                       [[1024, 128], [1, 128]])
                        nc.tensor.matmul(p2[:, i2a * 128:(i2a + 1) * 128],
                                         lhsT, w3s, start=True, stop=True)
                    # psum pos (i2a, n2=(i3a,b,i3b)); merge (b,i3b)->[32,16]
                    if j0 == 31:
                        # Barrier split: phase II's first matmul group needs
                        # only b1's (k=0, i3a 0-1) region. Land that 32-col
                        # micro-piece first on the (idle) DVE, then the rest
                        # on Act, so stage 3 starts ~1us earlier.
                        evac('d', bass.AP(b1_t, b1_o + j0,
                                          [[32768, 128], [512, 2], [32, 16]]),
                             bass.AP(p2.tensor, p2.offset,
                                     [[1024, 128], [1, 32]]))
                        evac('d', bass.AP(b1_t, b1_o + j0 + 1024,
                                          [[32768, 128], [512, 6], [32, 16]]),
                             bass.AP(p2.tensor, p2.offset + 32,
                                     [[1024, 128], [1, 96]]))
                        evac('a', bass.AP(b1_t, b1_o + j0 + 4096,
                                          [[32768, 128], [4096, 7], [512, 8], [32, 16]]),
                             bass.AP(p2.tensor, p2.offset + 128,
                                     [[1024, 128], [1, 896]]))
                    else:
                        evac(e2, bass.AP(b1_t, b1_o + j0,
                                         [[32768, 128], [4096, 8], [512, 8], [32, 16]]),
                             p2[:])

            # ---- Phase II: S3 (contract j1) + S4 (contract j0), per i2a ----
            with tc.tile_pool(name="t3p", bufs=4) as t3p, \
                 tc.tile_pool(name="stgp", bufs=3) as stgp, \
                 tc.tile_pool(name="ps3", bufs=2, space="PSUM") as ps3, \
                 tc.tile_pool(name="ps4", bufs=2, space="PSUM") as ps4:
                for k in range(8):  # k = i2a
                    # T3: [part (i3b,j0), free (i3a:512, b:128, (i1*4+i2b):1)]
                    t3 = t3p.tile([128, 4096], BF16, name="t3")
                    t3_t, t3_o = t3.tensor, t3.offset
                    for th in range(4):  # pairs of i3a
                        # At the three Act-skewed positions, borrow the psum
                        # tile from the ps4 pool: the late-returning Act evac
                        # then blocks ps4's rotation (slack there) instead of
                        # stalling the DVE-paced S3 stream via ps3 reuse.
                        if (4 * k + th) in (2, 10, 22):
                            p3 = ps4.tile([128, 1024], F32, name="p4")
                        else:
                            p3 = ps3.tile([128, 1024], F32, name="p3")
                        for q in range(8):
                            cq = 8 * th + q      # cq = i3a*4 + b
                            lhsT = bass.AP(b1_t,
                                           b1_o + k * 4096 + cq * 128,
                                           [[32768, 128], [1, 128]])
                            nc.tensor.matmul(p3[:, q * 128:(q + 1) * 128],
                                             lhsT, w1s, start=True, stop=True)
                        # S3 evacs on DVE except three positions skewed to
                        # Act (empirically best load-balance points).
                        evac('a' if (4 * k + th) in (2, 10, 22) else 'd',
                             t3[:, th * 1024:(th + 1) * 1024], p3[:])

                    stg = stgp.tile([128, 4096], BF16, name="stg")
                    stg_t, stg_o = stg.tensor, stg.offset
                    for th in range(4):  # pairs of i3a
                        p4 = ps4.tile([128, 1024], F32, name="p4")
                        for m in range(2):
                            i3a = 2 * th + m
                            rhs = bass.AP(t3_t, t3_o + i3a * 512,
                                          [[4096, 128], [128, 4], [1, 128]])
                            nc.tensor.matmul(p4[:, m * 512:(m + 1) * 512],
                                             w0s, rhs, start=True, stop=True)
                        # S4 evacs on Act, except (7,2) handed to DVE:
                        # DVE finishes its S3 stream ~3us before Act drains
                        # its k=7 backlog, so it absorbs one late unit.
                        evac('d' if (k, th) == (7, 2) else 'a',
                             stg[:, th * 1024:(th + 1) * 1024], p4[:])
                        nc.sync.dma_start(
                            out=bass.AP(y, k * 524288 + th * 1024,
                                        [[4096, 128], [1, 1024]]),
                            in_=bass.AP(stg_t, stg_o + th * 1024,
                                        [[4096, 128], [1, 1024]]))

    nc.finalize()
    return nc


def _build_waug(w: np.ndarray, kind: str) -> np.ndarray:
    """Augmented 128x128 weights (see baseline docstring)."""
    wa = np.zeros((128, 128), dtype=np.float32)
    ar = np.arange(32)
    if kind == "w3":
        # rows p = j3*4 + b ; cols n = i3a*16 + b*4 + i3b
        for b in range(4):
            cols = (ar >> 2) * 16 + b * 4 + (ar & 3)
            wa[np.ix_(ar * 4 + b, cols)] = w.T
    else:
        # rows p = q*32 + j ; cols n = i*4 + q
        for q in range(4):
            wa[np.ix_(q * 32 + ar, ar * 4 + q)] = w.T
    return wa


def _get_nc():
    if "nc" not in _NC_CACHE:
        _NC_CACHE["nc"] = _build_nc()
    return _NC_CACHE["nc"]


def make_in_maps(x, W0, W1, W2, W3):
    x = np.asarray(x, dtype=np.float32)
    bf = ml_dtypes.bfloat16
    wblock = np.concatenate([
        _build_waug(np.asarray(W2, np.float32), "q"),
        _build_waug(np.asarray(W3, np.float32), "w3"),
        _build_waug(np.asarray(W1, np.float32), "q"),
        _build_waug(np.asarray(W0, np.float32), "q"),
    ], axis=1).astype(bf)
    xr = x.reshape(32, 4, 8, 32, 32, B)
    in_maps = []
    for c in range(NCORES):
        xc = xr[..., c * BC:(c + 1) * BC].transpose(0, 1, 3, 2, 4, 5)
        xc = np.ascontiguousarray(xc).astype(bf).reshape(32, 131072)
        x0 = xc[0].reshape(128, 1024)
        wpack = np.concatenate([wblock[:, 0:128], x0[:, 0:512],
                                wblock[:, 128:512], x0[:, 512:1024]], axis=1)
        in_maps.append({"x": xc, "wpack": wpack})
    return in_maps


def _unshuffle_y(yd: np.ndarray) -> np.ndarray:
    """[i2a(8), (i0, i3b), (i3a, b, i1, i2b)] -> [N, BC]."""
    y = yd.astype(np.float32).reshape(8, 32, 4, 8, BC, 32, 4)
    y = y.transpose(1, 5, 0, 6, 3, 2, 4)
    return np.ascontiguousarray(y).reshape(N, BC)


def kernel(x, W0, W1, W2, W3, _trace=False):
    nc = _get_nc()
    in_maps = make_in_maps(x, W0, W1, W2, W3)
    res = run_bass_kernel_spmd(nc, in_maps, core_ids=list(range(NCORES)),
                               trace=_trace)
    out = np.concatenate(
        [_unshuffle_y(res.results[c]["y"]) for c in range(NCORES)], axis=1)
    if _trace:
        kernel.last_result = res
    return out


if __name__ == "__main__":
    rng = np.random.default_rng(0)
    x = rng.standard_normal((N, B), dtype=np.float32)
    ws = [rng.standard_normal((L, L), dtype=np.float32) for _ in range(4)]
    y = kernel(x, *ws)
    print("ran", y.shape, y.dtype)
